# revision 1
# baseline (speedup 1.0000x reference)
"""Trainium2 Bass kernel for a 3-layer PointGNN-style edge-scored message-passing GNN.

See reference semantics in the problem statement. Strategy:

Host folds weights so that per layer:
    x' = X@Wx + bx ; B = X@Wb + bb ; A = X@Wa + ba
    h = relu(A[dst] + B[src]) ; s = sigmoid(h.w2 + b2)
    out[d] = sum_{e: dst=d} s_e * x'[src_e]       (+relu for layers 1,2)

Device (8-core SPMD, dst-partitioned, one identical program; all per-core
variation comes through input data):
  - nodes permuted into nb bins of 128 slots, balanced by in-degree; every
    bin's (edges+self-loops) group is padded to one uniform count g_pad.
  - node phase (all bins, redundant per core): matmul chains from feature-major
    X produce a DRAM row table [x'|B] ([ng, 2*co]); A tiles for own bins in SBUF.
  - edge phase (own 30 bins): dma_gather [x'|B] rows by src index; one-hot
    matmuls for A[dst] gather and dst scatter-add (PSUM accumulate); fused
    relu*w2 row-reduction (scalar_tensor_tensor accum_out) for the edge score.
  - layers 1,2 emit output feature-major ([64, 3840]) via the transposed
    scatter matmul; AllGather moves it to every core for the next layer.
"""

import sys

if "/opt/trn_rl_repo" not in sys.path:
    sys.path.insert(0, "/opt/trn_rl_repo")

import numpy as np

import concourse.bacc as bacc
import concourse.bass as bass  # noqa: F401
import concourse.mybir as mybir
import concourse.tile as tile
from concourse.bass_utils import run_bass_kernel_spmd

F32 = mybir.dt.float32
I16 = mybir.dt.int16
AF = mybir.ActivationFunctionType
ALU = mybir.AluOpType

P = 128
NCORES = 8
GCH_TILES = 0   # gather chunk size in 128-edge tiles; 0 = half-bin
SIM_MODE = False  # replace collectives with local copies (TimelineSim support)


class Cfg:
    def __init__(self, n_real, nbc, dims):
        self.n_real = n_real
        self.nbc = nbc
        self.nb = nbc * NCORES
        self.ng = self.nb * P
        self.dims = dims


CFG = Cfg(30000, 30, [(256, 64), (64, 64), (64, 256)])


# ---------------------------------------------------------------- host prep

def _balance_bins(weight, nb):
    """Assign nodes to nb bins of <=128 slots, balancing sum(weight)."""
    import heapq

    n = weight.shape[0]
    order = np.argsort(-weight, kind="stable")
    bin_of = np.empty(n, np.int32)
    slot_of = np.empty(n, np.int32)
    counts = np.zeros(nb, np.int32)
    heap = [(0, b) for b in range(nb)]
    heapq.heapify(heap)
    for i in order:
        spill = []
        while True:
            load, b = heapq.heappop(heap)
            if counts[b] < P:
                break
            spill.append((load, b))
        for s in spill:
            heapq.heappush(heap, s)
        bin_of[i] = b
        slot_of[i] = counts[b]
        counts[b] += 1
        heapq.heappush(heap, (load + int(weight[i]), b))
    return bin_of, slot_of


def _wrap16(flat_idx):
    n = flat_idx.shape[0]
    a = flat_idx.reshape(n // 16, 16).T.astype(np.int16)
    return np.tile(a, (8, 1))


def _host_prep(x, src, dst, cfg):
    n = cfg.n_real
    loops = np.arange(n, dtype=np.int64)
    src_all = np.concatenate([src, loops])
    dst_all = np.concatenate([dst, loops])

    indeg = np.bincount(dst_all, minlength=n).astype(np.int64)
    bin_of, slot_of = _balance_bins(indeg, cfg.nb)
    g_of = bin_of.astype(np.int64) * P + slot_of

    e_bin = bin_of[dst_all]
    order = np.argsort(e_bin, kind="stable")
    sb = e_bin[order]
    counts = np.bincount(e_bin, minlength=cfg.nb)
    g_pad = int(np.ceil(max(counts.max(), 1) / P) * P)
    starts = np.zeros(cfg.nb, np.int64)
    starts[1:] = np.cumsum(counts)[:-1]
    rank = np.arange(sb.shape[0]) - starts[sb]

    src_g = np.zeros((cfg.nb, g_pad), np.int64)             # pad edges -> row 0
    dst_slot = np.full((cfg.nb, g_pad), 255.0, np.float32)  # pad -> no match
    src_g[sb, rank] = g_of[src_all[order]]
    dst_slot[sb, rank] = slot_of[dst_all[order]].astype(np.float32)

    nt_e = g_pad // P
    per_core = []
    for c in range(NCORES):
        bins = slice(c * cfg.nbc, (c + 1) * cfg.nbc)
        sg = src_g[bins]
        srcw = np.concatenate([_wrap16(sg[t]) for t in range(cfg.nbc)], axis=1)
        dc = dst_slot[bins]
        dstc = np.concatenate(
            [dc[t].reshape(nt_e, P).T for t in range(cfg.nbc)], axis=1
        ).astype(np.float32)
        per_core.append((srcw, dstc))

    c_in = cfg.dims[0][0]
    x1t = np.zeros((c_in, cfg.ng), np.float32)
    x1t[:, g_of] = x.T
    return g_of, g_pad, per_core, x1t


def _fuse_weights(ws, cfg):
    out = []
    for li, (ci, co) in enumerate(cfg.dims, start=1):
        wl = ws[f"w_lin{li}"].astype(np.float64)
        bl = ws[f"b_lin{li}"].astype(np.float64)
        ws1 = ws[f"w_s1_{li}"].astype(np.float64)
        bs1 = ws[f"b_s1_{li}"].astype(np.float64)
        ws2 = ws[f"w_s2_{li}"].astype(np.float64)
        bs2 = ws[f"b_s2_{li}"].astype(np.float64)
        wi, wj = ws1[:co], ws1[co:]
        wmat = np.zeros((ci + 1, 3 * co), np.float32)
        wmat[:ci, :co] = wl
        wmat[ci, :co] = bl
        wmat[:ci, co : 2 * co] = wl @ wj
        wmat[ci, co : 2 * co] = bl @ wj
        wmat[:ci, 2 * co :] = wl @ wi
        wmat[ci, 2 * co :] = bl @ wi + bs1
        out.append(dict(wmat=wmat, w2=ws2[:, 0].astype(np.float32), b2=np.float32(bs2[0])))
    return out


# ---------------------------------------------------------------- program

def _build_program(cfg, g_pad):
    nbc, ng = cfg.nbc, cfg.ng
    nt_e = g_pad // P
    dims = cfg.dims
    nl = len(dims)
    c_in1 = dims[0][0]
    c_out_last = dims[-1][1]
    n_loc = nbc * P
    k1 = c_in1 // P                 # lhsT k-chunks for layer 1
    nb1 = max(1, nbc // 2)          # bins per L1 node super-chunk
    nch1 = (cfg.nb + nb1 - 1) // nb1
    jh = GCH_TILES if GCH_TILES > 0 else (nt_e + 1) // 2  # tiles per gather chunk

    # const blob [128, cb_cols]; row 0 carries ones_row in its column range
    c_iota, c_iotac, c_ident = 0, 128, 129
    off = 257
    c_w2 = []
    for l in range(nl):
        c_w2.append(off)
        off += dims[l][1]
    c_b2 = list(range(off, off + nl))
    off += nl
    c_ones = off
    off += 128
    cb_cols = off

    nc = bacc.Bacc("TRN2", target_bir_lowering=False, debug=False, num_devices=NCORES)

    x1t_d = nc.dram_tensor("x1t", [c_in1, ng], F32, kind="ExternalInput")
    xa1_d = nc.dram_tensor("xa1", [c_in1, n_loc], F32, kind="ExternalInput")
    cst_d = nc.dram_tensor("cst", [P, cb_cols], F32, kind="ExternalInput")
    srcw_d = nc.dram_tensor("srcw", [P, nbc * g_pad // 16], I16, kind="ExternalInput")
    dstc_d = nc.dram_tensor("dstc", [P, nbc * nt_e], F32, kind="ExternalInput")
    w_d = [
        nc.dram_tensor(f"w{l + 1}", [dims[l][0] + 1, 3 * dims[l][1]], F32, kind="ExternalInput")
        for l in range(nl)
    ]
    out_d = nc.dram_tensor("out", [n_loc, c_out_last], F32, kind="ExternalOutput")

    with tile.TileContext(nc) as tc:
        with (
            tc.tile_pool(name="cst", bufs=1) as cpool,
            tc.tile_pool(name="persist", bufs=1) as ppool,
            tc.tile_pool(name="xch", bufs=2) as xpool,
            tc.tile_pool(name="xa", bufs=2) as xapool,
            tc.tile_pool(name="work", bufs=3) as wpool,
            tc.tile_pool(name="gath", bufs=3) as gpool,
            tc.tile_pool(name="ps", bufs=2, space="PSUM") as pspool,
            tc.tile_pool(name="acc", bufs=2, space="PSUM") as accpool,
            tc.tile_pool(name="dram", bufs=1, space="DRAM") as dpool,
        ):
            # ---------------- constants
            cst = cpool.tile([P, cb_cols], F32)
            nc.sync.dma_start(cst[:], cst_d[:])
            srcw = cpool.tile([P, nbc * g_pad // 16], I16)
            nc.sync.dma_start(srcw[:], srcw_d[:])
            dstc = cpool.tile([P, nbc * nt_e], F32)
            nc.sync.dma_start(dstc[:], dstc_d[:])
            wfeat = []   # per layer: list of [128-or-64, 3co] SBUF tiles (k-chunks)
            wbias = []   # per layer: [1, 3co] bias-row AP
            for l in range(nl):
                ci_l, co_l = dims[l]
                nk = (ci_l + P - 1) // P
                chunks = []
                for k in range(nk):
                    r0, r1 = k * P, min((k + 1) * P, ci_l)
                    w_t = cpool.tile([r1 - r0, 3 * co_l], F32, tag=f"w{l}_{k}")
                    nc.sync.dma_start(w_t[:], w_d[l][:][r0:r1, :])
                    chunks.append(w_t)
                wb = cpool.tile([1, 3 * co_l], F32, tag=f"w{l}_b")
                nc.sync.dma_start(wb[:], w_d[l][:][ci_l : ci_l + 1, :])
                wfeat.append(chunks)
                wbias.append(wb)

            iota_free = cst[:, c_iota : c_iota + 128]
            iota_col = cst[:, c_iotac : c_iotac + 1]
            ident = cst[:, c_ident : c_ident + 128]
            ones_row = cst[0:1, c_ones : c_ones + 128]

            # ---------------- persistent
            a_sb = ppool.tile([P, nbc * max(d[1] for d in dims)], F32, tag="a_sb")
            xloc = ppool.tile([64, n_loc], F32, tag="xloc")

            # ---------------- DRAM internals
            tables = [dpool.tile([ng, 2 * dims[l][1]], F32, tag=f"table{l}", name=f"table{l}") for l in range(nl)]
            ag_in = [dpool.tile([64, n_loc], F32, tag=f"agin{l}", name=f"agin{l}") for l in range(nl - 1)]
            ag_out = [
                dpool.tile(
                    [NCORES * 64, n_loc],
                    F32,
                    tag=f"agout{l}",
                    name=f"agout{l}",
                    addr_space="Local" if SIM_MODE else "Shared",
                )
                for l in range(nl - 1)
            ]

            for l in range(nl):
                ci, co = dims[l]
                table = tables[l]

                # ======== node phase: all nb bins, streamed in super-chunks
                if l == 0:
                    chunks = [
                        (r * nb1, min(nb1, cfg.nb - r * nb1)) for r in range(nch1)
                    ]
                else:
                    chunks = [(r * nbc, nbc) for r in range(NCORES)]
                for b0, nbch in chunks:
                    w_ch = nbch * P
                    if l == 0:
                        xch = xpool.tile([P, k1 * nb1 * P], F32, tag="xch")
                        xch3 = xch[:, 0 : k1 * w_ch].rearrange(
                            "p (c n) -> p c n", c=k1
                        )
                        nc.sync.dma_start(
                            xch3,
                            x1t_d[:, b0 * P : b0 * P + w_ch].rearrange(
                                "(c p) n -> p c n", p=P
                            ),
                        )
                        kch = [(xch3[:, k, :], wfeat[l][k]) for k in range(k1)]
                    else:
                        xch = xpool.tile([64, nbc * P], F32, tag="xch")
                        nc.sync.dma_start(
                            xch[:, 0:w_ch],
                            ag_out[l - 1][:][(b0 // nbc) * 64 : (b0 // nbc) * 64 + 64, :],
                        )
                        kch = [(xch[:, 0:w_ch], wfeat[l][0])]

                    for t in range(nbch):
                        b = b0 + t
                        cols = slice(t * P, (t + 1) * P)
                        xb_ps = pspool.tile([P, 2 * co], F32, space="PSUM", tag="psA")
                        for k, (kc, wt) in enumerate(kch):
                            nc.tensor.matmul(
                                out=xb_ps[:],
                                lhsT=kc[:, cols],
                                rhs=wt[:, 0 : 2 * co],
                                start=(k == 0),
                                stop=False,
                            )
                        nc.tensor.matmul(
                            out=xb_ps[:],
                            lhsT=ones_row,
                            rhs=wbias[l][0:1, 0 : 2 * co],
                            start=False,
                            stop=True,
                        )
                        xb_sb = wpool.tile([P, 2 * co], F32, tag="xb_sb")
                        nc.scalar.activation(out=xb_sb[:], in_=xb_ps[:], func=AF.Copy)
                        nc.sync.dma_start(table[:][b * P : (b + 1) * P, :], xb_sb[:])

                # ======== A phase: own bins (per-core lhsT inputs)
                for t in range(nbc):
                    cols = slice(t * P, (t + 1) * P)
                    a_ps = pspool.tile([P, co], F32, space="PSUM", tag="psB")
                    if l == 0:
                        xa = xapool.tile([P, k1 * P], F32, tag="xa")
                        xa3 = xa[:].rearrange("p (c n) -> p c n", c=k1)
                        nc.sync.dma_start(
                            xa3,
                            xa1_d[:, t * P : (t + 1) * P].rearrange(
                                "(c p) n -> p c n", p=P
                            ),
                        )
                        for k in range(k1):
                            nc.tensor.matmul(
                                out=a_ps[:],
                                lhsT=xa3[:, k, :],
                                rhs=wfeat[l][k][:, 2 * co : 3 * co],
                                start=(k == 0),
                                stop=False,
                            )
                    else:
                        nc.tensor.matmul(
                            out=a_ps[:],
                            lhsT=xloc[:, cols],
                            rhs=wfeat[l][0][:, 2 * co : 3 * co],
                            start=True,
                            stop=False,
                        )
                    nc.tensor.matmul(
                        out=a_ps[:],
                        lhsT=ones_row,
                        rhs=wbias[l][0:1, 2 * co : 3 * co],
                        start=False,
                        stop=True,
                    )
                    nc.scalar.activation(
                        out=a_sb[:, t * co : (t + 1) * co], in_=a_ps[:], func=AF.Copy
                    )

                # ======== edge phase: own bins, gathers in flat 1024-idx chunks
                w2rep = cst[:, c_w2[l] : c_w2[l] + co]
                b2col = cst[:, c_b2[l] : c_b2[l] + 1]
                n_gt = nbc * nt_e
                CH = 8  # tiles per gather chunk (<=1024 idxs: SWDGE packet cap)
                g3 = None
                o_ps = s_pre = s_sig = scr = None
                for gt in range(n_gt):
                    t, j = divmod(gt, nt_e)
                    if gt % CH == 0:
                        hn = min(CH, n_gt - gt)
                        gbuf = gpool.tile([P, CH * 2 * co], F32, tag="gbuf")
                        g3 = gbuf[:, 0 : hn * 2 * co].rearrange(
                            "p (j d) -> p j d", d=2 * co
                        )
                        nc.gpsimd.dma_gather(
                            out_ap=g3,
                            in_ap=table[:],
                            idxs_ap=srcw[:, gt * 8 : (gt + hn) * 8],
                            num_idxs=hn * P,
                            num_idxs_reg=hn * P,
                            elem_size=2 * co,
                        )
                    slot = gt % CH
                    if j == 0:
                        if l < nl - 1:
                            o_ps = accpool.tile([64, P], F32, space="PSUM", tag="o_ps")
                        else:
                            o_ps = accpool.tile([P, co], F32, space="PSUM", tag="o_ps")
                        s_pre = wpool.tile([P, nt_e], F32, tag="s_pre")
                        s_sig = wpool.tile([P, nt_e], F32, tag="s_sig")
                        scr = wpool.tile([P, co], F32, tag="scr")
                    oh = wpool.tile([P, P], F32, tag="oh")
                    nc.vector.tensor_tensor(
                        out=oh[:],
                        in0=dstc[:, t * nt_e + j : t * nt_e + j + 1].to_broadcast([P, P]),
                        in1=iota_free,
                        op=ALU.is_equal,
                    )
                    ohT_ps = pspool.tile([P, P], F32, space="PSUM", tag="psB")
                    nc.tensor.transpose(out=ohT_ps[:], in_=oh[:], identity=ident)
                    oht = wpool.tile([P, P], F32, tag="oht")
                    nc.vector.tensor_copy(out=oht[:], in_=ohT_ps[:])
                    ab = pspool.tile([P, co], F32, space="PSUM", tag="psA")
                    nc.tensor.matmul(
                        out=ab[:],
                        lhsT=oht[:],
                        rhs=a_sb[:, t * co : (t + 1) * co],
                        start=True,
                        stop=False,
                    )
                    nc.tensor.matmul(
                        out=ab[:],
                        lhsT=ident,
                        rhs=g3[:, slot, co : 2 * co],
                        start=False,
                        stop=True,
                    )
                    nc.vector.scalar_tensor_tensor(
                        out=scr[:],
                        in0=ab[:],
                        scalar=0.0,
                        in1=w2rep,
                        op0=ALU.max,
                        op1=ALU.mult,
                        accum_out=s_pre[:, j : j + 1],
                    )
                    nc.scalar.activation(
                        out=s_sig[:, j : j + 1],
                        in_=s_pre[:, j : j + 1],
                        func=AF.Sigmoid,
                        bias=b2col,
                    )
                    msg = wpool.tile([P, co], F32, tag="msg")
                    nc.scalar.activation(
                        out=msg[:],
                        in_=g3[:, slot, 0:co],
                        func=AF.Copy,
                        scale=s_sig[:, j : j + 1],
                    )
                    if l < nl - 1:
                        nc.tensor.matmul(
                            out=o_ps[:],
                            lhsT=msg[:],
                            rhs=oh[:],
                            start=(j == 0),
                            stop=(j == nt_e - 1),
                        )
                    else:
                        nc.tensor.matmul(
                            out=o_ps[:],
                            lhsT=oh[:],
                            rhs=msg[:],
                            start=(j == 0),
                            stop=(j == nt_e - 1),
                        )
                    if j == nt_e - 1:
                        if l < nl - 1:
                            nc.scalar.activation(
                                out=xloc[:, t * P : (t + 1) * P],
                                in_=o_ps[:],
                                func=AF.Relu,
                            )
                        else:
                            ostg = wpool.tile([P, co], F32, tag="ostg")
                            nc.scalar.activation(out=ostg[:], in_=o_ps[:], func=AF.Copy)
                            nc.sync.dma_start(out_d[t * P : (t + 1) * P, :], ostg[:])

                # ======== allgather (layers 0..nl-2)
                if l < nl - 1:
                    nc.sync.dma_start(ag_in[l][:], xloc[:])
                    if SIM_MODE:
                        for r in range(NCORES):
                            nc.sync.dma_start(
                                ag_out[l][:][r * 64 : (r + 1) * 64, :], ag_in[l][:]
                            )
                    else:
                        nc.gpsimd.collective_compute(
                            "AllGather",
                            ALU.bypass,
                            replica_groups=[list(range(NCORES))],
                            ins=[ag_in[l].opt()],
                            outs=[ag_out[l].opt()],
                        )

    nc.compile()
    return nc


# ---------------------------------------------------------------- driver

_PROG_CACHE = {}


def _run(inputs, cfg, trace=False):
    x = np.ascontiguousarray(np.asarray(inputs["x"], dtype=np.float32))
    ei = np.asarray(inputs["edge_index"]).astype(np.int64)
    src, dst = ei[0], ei[1]

    g_of, g_pad, per_core, x1t = _host_prep(x, src, dst, cfg)
    fw = _fuse_weights(inputs, cfg)

    key = (cfg.n_real, cfg.nbc, g_pad)
    if key not in _PROG_CACHE:
        _PROG_CACHE[key] = _build_program(cfg, g_pad)
    nc = _PROG_CACHE[key]

    nbc, nl = cfg.nbc, len(cfg.dims)
    n_loc = nbc * P
    c_w2_w = sum(d[1] for d in cfg.dims)
    cb_cols = 257 + c_w2_w + nl + 128
    cst = np.zeros((P, cb_cols), np.float32)
    cst[:, 0:128] = np.arange(128, dtype=np.float32)[None, :]
    cst[:, 128] = np.arange(128, dtype=np.float32)
    cst[:, 129:257] = np.eye(128, dtype=np.float32)
    off = 257
    for l in range(nl):
        cst[:, off : off + cfg.dims[l][1]] = fw[l]["w2"][None, :]
        off += cfg.dims[l][1]
    for l in range(nl):
        cst[:, off] = fw[l]["b2"]
        off += 1
    cst[0, off : off + 128] = 1.0

    in_maps = []
    for c in range(NCORES):
        srcw, dstc_a = per_core[c]
        in_maps.append(
            {
                "x1t": x1t,
                "xa1": np.ascontiguousarray(x1t[:, c * n_loc : (c + 1) * n_loc]),
                "cst": cst,
                "srcw": srcw,
                "dstc": dstc_a,
                **{f"w{l + 1}": fw[l]["wmat"] for l in range(nl)},
            }
        )

    res = run_bass_kernel_spmd(nc, in_maps, core_ids=list(range(NCORES)), trace=trace)

    full = np.empty((cfg.ng, cfg.dims[-1][1]), np.float32)
    for c in range(NCORES):
        full[c * n_loc : (c + 1) * n_loc] = res.results[c]["out"]
    out = full[g_of]
    return out, res


def kernel(**inputs) -> np.ndarray:
    out, _ = _run(inputs, CFG, trace=False)
    return out

